# revision 6
# baseline (speedup 1.0000x reference)
"""nn_BayesianLayer — reparameterized Bayesian linear layer + inverted dropout
on 8 TRN2 NeuronCores (data-parallel over the 65536-row batch).

reference:
  w = w_mu + softplus(w_rho) * w_eps            [512, 512]
  b = b_mu + softplus(b_rho) * b_eps            [512]
  y = (x @ w.T + b) * (drop_u >= 0.2) / 0.8     [65536, 512]

Sharding: x and drop_u split into 8 row-shards of 8192; the small weight
tensors are replicated. Each core runs the same single-core Bass/Tile graph
(SPMD, no collectives); outputs are concatenated on the host.

The kernel is purely HBM-bandwidth-bound (48 MiB/core steady-state: x and
drop_u in, y out; pure-DMA probes of the same pattern run no faster than the
full kernel). With all 8 cores streaming, each pair of cores shares one HBM
stack, capping per-core bandwidth at ~300-315 GB/s (vs ~358 for a lone
core), so the only real lever is DMA descriptor-line size:

 - batch rows are assigned to partitions P-MAJOR: partition p owns rows
   [64p, 64p+64). Output tile t computes row 64p+t on partition p, so
   drop_u and y need NO host-side reordering at all (a [8192,512] row-major
   array IS p-major: "(p t) n -> p (t n)"), and per-partition DMA lines for
   du/y are 16 KiB (8 rows) instead of 2 KiB. x is host-transposed (the
   TensorEngine contracts over partitions; fp32 DMA-transpose doesn't
   exist) with columns interleaved to (t*128+p) order so x slab loads are
   8 KiB lines. Measured on pure-DMA probes: 2K/1K lines ~170 us/rep,
   8K/16K lines ~161-163 us/rep for the same 48 MiB.
 - slabs: x loaded 4 MiB per DMA (2 row-groups), du/y 2 MiB per DMA
   (1 row-group of 1024 rows); transfers alternate between the two HWDGE
   rings (SP + ACT) so per-2-group traffic is 6 MiB on each ring.

Per-core kernel design:
 - prologue computes w'T = 1.25*(w_mu + softplus(w_rho)*w_eps).T entirely
   on-device. softplus is relu(x) + ln1p(exp(-|x|)) with a 6-term
   polynomial for ln1p (this toolchain's ACT tables lack Softplus/Ln);
   the 1.25 dropout scale is folded into w', b'. It is emitted per k-chunk
   with the tensor_tensor tail ops on GPSIMD so the serial DVE chain that
   gates the first matmul stays short.
 - the bias is added via an extra K=1 matmul (ones[1,128].T @ b'[1,512])
   that initializes each PSUM accumulation group.
 - main loop: per 128-row tile 5 fp32r matmuls accumulate in one PSUM bank
   and a single fused DVE op applies the dropout mask:
   out = (drop_u >= 0.2) * psum.
 - matmul inputs are fp32r (TensorEngine fast-fp32 mode, 1 cycle/row at
   N=512 vs 4 for plain fp32); measured end-to-end rel err ~1.5e-4.
"""

import numpy as np

import concourse.bass as bass
import concourse.mybir as mybir
from concourse import bacc, tile
from concourse.bass import ts
from concourse.bass_utils import run_bass_kernel_spmd

AF = mybir.ActivationFunctionType
ALU = mybir.AluOpType

N_CORES = 8
B, IN, OUT = 65536, 512, 512
BS = B // N_CORES          # 8192 rows per core
P = 128
KC = IN // P               # 4 contraction chunks
GROUPS = 8                 # 1024-row groups per core
XSPAN = 2                  # groups per x slab (4 MiB DMAs, 8 KiB lines)
DROP = 0.2
SCALE = 1.0 / (1.0 - DROP)

# ln(1+t) ~= sum_{k=1..6} LN1P_COEF[k-1] * t^k on t in [0,1]  (max err 1.8e-6)
LN1P_COEF = [0.9998889, -0.49770296, 0.31687787, -0.19223858, 0.08419863,
             -0.017877892]


def build_kernel(x_bufs=2, du_bufs=3, out_bufs=3, psum_bufs=4, reps=1,
                 unroll=1):
    nc = bacc.Bacc(None, target_bir_lowering=False, debug=False)
    f32 = mybir.dt.float32
    f32r = mybir.dt.float32r
    gb = BS // GROUPS          # 1024 rows per group
    jt = gb // P               # 8 output tiles per group

    xt = nc.declare_dram_parameter("xt", [IN, BS], f32, isOutput=False)
    wmu = nc.declare_dram_parameter("wmu", [IN, OUT], f32, isOutput=False)
    wrho = nc.declare_dram_parameter("wrho", [IN, OUT], f32, isOutput=False)
    weps = nc.declare_dram_parameter("weps", [IN, OUT], f32, isOutput=False)
    bmu = nc.declare_dram_parameter("bmu", [1, OUT], f32, isOutput=False)
    brho = nc.declare_dram_parameter("brho", [1, OUT], f32, isOutput=False)
    beps = nc.declare_dram_parameter("beps", [1, OUT], f32, isOutput=False)
    du = nc.declare_dram_parameter("du", [BS, OUT], f32, isOutput=False)
    y = nc.declare_dram_parameter("y", [BS, OUT], f32, isOutput=True)

    # xt columns are host-interleaved: column t*128+p = x row 64p+t, so a
    # slab of 2048 consecutive columns is 8 KiB per partition line and
    # output tile t lands row 64p+t on partition p.
    xt_r = xt[:, :].rearrange("(k p) b -> p k b", p=P)            # [128, KC, BS]
    wmu_r = wmu[:, :].rearrange("(k p) n -> p k n", p=P)          # [128, KC, OUT]
    wrho_r = wrho[:, :].rearrange("(k p) n -> p k n", p=P)
    weps_r = weps[:, :].rearrange("(k p) n -> p k n", p=P)
    # du/y p-major: partition p owns rows [64p, 64p+64); group g is the
    # 8-row block [8g, 8g+8) of each partition -> 16 KiB contiguous lines.
    du_r = du[:, :].rearrange("(p g j) n -> p g (j n)", p=P, g=GROUPS)
    y_r = y[:, :].rearrange("(p g j) n -> p g (j n)", p=P, g=GROUPS)

    with tile.TileContext(nc) as tc:
        with (
            tc.tile_pool(name="wt", bufs=1) as wt_pool,
            tc.tile_pool(name="prol", bufs=2) as prol_pool,
            tc.tile_pool(name="bias", bufs=1) as bias_pool,
            tc.tile_pool(name="xs", bufs=x_bufs) as x_pool,
            tc.tile_pool(name="dus", bufs=du_bufs) as du_pool,
            tc.tile_pool(name="outs", bufs=out_bufs) as out_pool,
            tc.tile_pool(name="ps", bufs=psum_bufs, space="PSUM") as psum_pool,
        ):
            def emit_softplus(sp, x_t, scratch):
                """sp = softplus(x_t) = relu(x) + ln1p(exp(-|x|))."""
                # scratch = exp(-|x|); |x| by clearing the sign bit (abs_max
                # is not in the DVE tensor_scalar ISA)
                nc.vector.tensor_scalar(
                    scratch[:].bitcast(mybir.dt.uint32),
                    x_t[:].bitcast(mybir.dt.uint32),
                    0x7FFFFFFF, None, ALU.bitwise_and)
                nc.scalar.activation(scratch[:], scratch[:], AF.Exp, scale=-1.0)
                # sp = poly(scratch): u = (u + a_k) * t, k = 8..1
                nc.vector.tensor_scalar_mul(sp[:], scratch[:], LN1P_COEF[-1])
                for a_k in reversed(LN1P_COEF[:-1]):
                    nc.vector.scalar_tensor_tensor(
                        sp[:], sp[:], a_k, scratch[:], ALU.add, ALU.mult)
                # scratch = relu(x); sp += scratch
                nc.scalar.activation(scratch[:], x_t[:], AF.Relu)
                nc.vector.tensor_add(sp[:], sp[:], scratch[:])

            # ---- weight prologue, per-k chunks: the first PSUM group needs
            # ALL of w', so total prologue latency gates the first matmul;
            # chunking pipelines ACT/DVE/GPSIMD and the 2-input tail ops run
            # on the otherwise-idle GPSIMD ----
            wt = []
            for k in range(KC):
                mu_t = prol_pool.tile([P, OUT], f32, tag="mu")
                rho_t = prol_pool.tile([P, OUT], f32, tag="rho")
                eps_t = prol_pool.tile([P, OUT], f32, tag="eps")
                nc.scalar.dma_start(out=rho_t[:], in_=wrho_r[:, k])
                nc.sync.dma_start(out=mu_t[:], in_=wmu_r[:, k])
                nc.sync.dma_start(out=eps_t[:], in_=weps_r[:, k])
                sp = prol_pool.tile([P, OUT], f32, tag="sp")
                scr = prol_pool.tile([P, OUT], f32, tag="scr")
                emit_softplus(sp, rho_t, scr)
                nc.gpsimd.tensor_mul(sp[:], sp[:], eps_t[:])
                nc.gpsimd.tensor_add(sp[:], sp[:], mu_t[:])
                wtk = wt_pool.tile([P, OUT], f32r, tag=f"wt{k}")
                nc.scalar.mul(wtk[:], sp[:], SCALE)
                wt.append(wtk)

            # ---- bias prologue: b' row [1, OUT], scaled by 1.25 ----
            bmu_t = bias_pool.tile([1, OUT], f32, tag="bmu")
            brho_t = bias_pool.tile([1, OUT], f32, tag="brho")
            beps_t = bias_pool.tile([1, OUT], f32, tag="beps")
            nc.scalar.dma_start(out=bmu_t[:], in_=bmu[:, :])
            nc.scalar.dma_start(out=brho_t[:], in_=brho[:, :])
            nc.scalar.dma_start(out=beps_t[:], in_=beps[:, :])
            spb = bias_pool.tile([1, OUT], f32, tag="spb")
            scrb = bias_pool.tile([1, OUT], f32, tag="scrb")
            emit_softplus(spb, brho_t, scrb)
            nc.vector.tensor_mul(spb[:], spb[:], beps_t[:])
            nc.vector.tensor_add(spb[:], spb[:], bmu_t[:])
            b_row = bias_pool.tile([1, OUT], f32r, tag="brow")
            nc.scalar.mul(b_row[:], spb[:], SCALE)
            # memset can't write fp32r; go through an f32 tile + ACT copy
            ones_t = bias_pool.tile([1, P], f32r, tag="ones")
            ones_f = bias_pool.tile([1, P], f32, tag="onesf")
            nc.vector.memset(ones_f[:], 1.0)
            nc.scalar.copy(ones_t[:], ones_f[:])

            # ---- main loop: x in 4 MiB 2-group slabs on SP; du/y 2 MiB
            # single-group transfers split so each ring moves 6 MiB per
            # 2-group span (SP: x + du[even]; ACT: du[odd] + both y) ----
            def main_body():
                for i in range(GROUPS // XSPAN):
                    xs = x_pool.tile([P, KC, XSPAN * gb], f32r, tag="xs")
                    nc.sync.dma_start(
                        out=xs[:],
                        in_=xt_r[:, :, i * XSPAN * gb:(i + 1) * XSPAN * gb]
                        .bitcast(f32r))
                    for h in range(XSPAN):
                        g = i * XSPAN + h
                        dus = du_pool.tile([P, jt, OUT], f32, tag="dus")
                        eng = nc.sync if h == 0 else nc.scalar
                        eng.dma_start(out=dus[:], in_=du_r[:, g])
                        outs = out_pool.tile([P, jt, OUT], f32, tag="outs")
                        for j in range(jt):
                            ps = psum_pool.tile([P, OUT], f32, tag="ps")
                            nc.tensor.matmul(
                                ps[:], ones_t[:], b_row[:],
                                start=True, stop=False)
                            for k in range(KC):
                                nc.tensor.matmul(
                                    ps[:], xs[:, k, ts(h * jt + j, P)], wt[k],
                                    start=False, stop=(k == KC - 1))
                            # out = (drop_u >= 0.2) * psum  (one fused DVE op)
                            nc.vector.scalar_tensor_tensor(
                                outs[:, j], dus[:, j], DROP, ps[:],
                                ALU.is_ge, ALU.mult)
                        nc.scalar.dma_start(out=y_r[:, g], in_=outs[:])

            if reps == 1 and unroll == 1:
                main_body()
            else:
                with tc.For_i(0, reps):
                    for _ in range(unroll):
                        main_body()

    nc.finalize()
    return nc


def shard_inputs(x, w_mu, w_rho, b_mu, b_rho, w_eps, b_eps, drop_u):
    """Full inputs -> per-core in_maps (host-side slicing + layout prep)."""
    wmu_t = np.ascontiguousarray(np.asarray(w_mu, np.float32).T)
    wrho_t = np.ascontiguousarray(np.asarray(w_rho, np.float32).T)
    weps_t = np.ascontiguousarray(np.asarray(w_eps, np.float32).T)
    bmu = np.asarray(b_mu, np.float32).reshape(1, OUT)
    brho = np.asarray(b_rho, np.float32).reshape(1, OUT)
    beps = np.asarray(b_eps, np.float32).reshape(1, OUT)
    x = np.asarray(x, np.float32)
    drop_u = np.asarray(drop_u, np.float32)
    rpp = BS // P              # rows per partition (p-major row order)
    in_maps = []
    for c in range(N_CORES):
        sl = slice(c * BS, (c + 1) * BS)
        # xt column t*128+p = x row 64p+t of this shard
        xt = np.ascontiguousarray(
            x[sl].T.reshape(IN, P, rpp).transpose(0, 2, 1).reshape(IN, BS))
        in_maps.append({
            "xt": xt,
            "wmu": wmu_t, "wrho": wrho_t, "weps": weps_t,
            "bmu": bmu, "brho": brho, "beps": beps,
            "du": np.ascontiguousarray(drop_u[sl]),
        })
    return in_maps


def kernel(x, w_mu, w_rho, b_mu, b_rho, w_eps, b_eps, drop_u):
    nc = build_kernel()
    in_maps = shard_inputs(x, w_mu, w_rho, b_mu, b_rho, w_eps, b_eps, drop_u)
    res = run_bass_kernel_spmd(nc, in_maps, core_ids=list(range(N_CORES)))
    # y is written p-major = natural row order; just concatenate shards
    return np.ascontiguousarray(
        np.concatenate([res.results[c]["y"] for c in range(N_CORES)], axis=0))
